# revision 1
# baseline (speedup 1.0000x reference)
"""Trainium2 Bass kernel for a diagonal-A linear dynamical system (LDS).

    Bu = inputs @ B            [B, T, S]
    h_t = h_{t-1} * A + Bu_t   (scan over T, diagonal A)
    y_t = h_t @ C              [B, T, O]

Shapes: inputs [16, 4096, 256], A [256], B [256, 256], C [256, 256],
h0 [256]; all float32.

Sharding: data-parallel over batch across 8 NeuronCores (2 batches per
core); A/B/C/h0 replicated.

Per-core dataflow (all tiles 128-partition):
  1. DMA u supertile [128t, 4sub, 256i] (natural layout, contiguous i).
  2. PE transpose 128x128 blocks -> uT [128i, 512t] in PSUM.
  3. ACT copies uT PSUM->SBUF (dtype knob applies here).
  4. PE matmul BuT[s, t] = B^T @ uT accumulated over i-halves into PSUM.
  5. DVE tensor_tensor_scan along t: state = A*state + Bu (exactly the
     recurrence; fp32 internal state), chained across supertiles via
     initial=prev last column. Output hT in SBUF.
  6. PE matmul y[t, o] = hT.T @ C (hT slices are the stationary operand).
  7. ACT copy y PSUM->SBUF, DMA out.
"""

import numpy as np

import concourse.bacc as bacc
import concourse.bass as bass
import concourse.mybir as mybir
import concourse.tile as tile
from concourse import bass_utils
from concourse.masks import make_identity

BATCH, T, D = 16, 4096, 256
NCORES = 8
BLOC = BATCH // NCORES  # batches per core
TT = 1024               # time supertile (DMA granularity)
NSUB = TT // 128        # 128-row subtiles per supertile
NJ = T // TT            # supertiles per sequence
SC = 512                # scan / PSUM chunk within a supertile
NTH = TT // SC          # chunks per supertile
F32 = mybir.dt.float32

# Matmul operand dtype knob: float32 (exact, 4 cyc/row), float32r (fp32
# data, 1 cyc/row at N>=256), bfloat16 (1 cyc/row, lossy).
MM_DT = mybir.dt.float32r

_CACHE: dict = {}


def _build_nc():
    nc = bacc.Bacc(trn_type="TRN2", target_bir_lowering=False)

    u = nc.dram_tensor("u", [BLOC, T, D], F32, kind="ExternalInput")
    Ad = nc.dram_tensor("A", [128, 2], F32, kind="ExternalInput")      # [s%128, s//128]
    Bd = nc.dram_tensor("B", [2, 128, D], MM_DT, kind="ExternalInput")  # [ihalf, i, s]
    Cd = nc.dram_tensor("C", [2, 128, D], MM_DT, kind="ExternalInput")  # [shalf, s, o]
    h0d = nc.dram_tensor("h0", [128, 2], F32, kind="ExternalInput")
    y = nc.dram_tensor("y", [BLOC, T, D], F32, kind="ExternalOutput")

    # t = j*TT + sub*128 + p
    u_r = u[:].rearrange("b (j s p) i -> b j p s i", p=128, s=NSUB)
    y_r = y[:].rearrange("b (j s p) o -> b j p s o", p=128, s=NSUB)

    mult = mybir.AluOpType.mult
    add = mybir.AluOpType.add

    with tile.TileContext(nc) as tc:
        with (
            tc.tile_pool(name="const", bufs=1) as const,
            tc.tile_pool(name="sbuf", bufs=3) as sbuf,
            tc.tile_pool(name="hpool", bufs=1) as hpool,
            tc.tile_pool(name="ps_ut", bufs=2, space="PSUM") as ps_ut,
            tc.tile_pool(name="ps_bu", bufs=2, space="PSUM") as ps_bu,
            tc.tile_pool(name="ps_y", bufs=3, space="PSUM") as ps_y,
        ):
            # --- constants ---
            ident = const.tile([128, 128], F32, name="ident")
            make_identity(nc, ident)

            A_col = const.tile([128, 2], F32, name="A_col")
            nc.sync.dma_start(A_col, Ad[:])
            h0c = const.tile([128, 2], F32, name="h0c")
            nc.sync.dma_start(h0c, h0d[:])

            ones = const.tile([128, SC], F32, name="ones")
            nc.vector.memset(ones, 1.0)
            A_bc = const.tile([128, 2, SC], F32, name="A_bc")
            for m in range(2):
                nc.scalar.mul(A_bc[:, m], ones, mul=A_col[:, m : m + 1])

            B_sb = const.tile([128, 2, D], MM_DT, name="B_sb")
            C_sb = const.tile([128, 2, D], MM_DT, name="C_sb")
            dma_w = (
                nc.gpsimd.dma_start
                if MM_DT == mybir.dt.bfloat16
                else nc.sync.dma_start
            )
            for k in range(2):
                dma_w(B_sb[:, k], Bd[k])
                dma_w(C_sb[:, k], Cd[k])

            # hidden states, [128s, b, mhalf, t]; persistent
            hT = hpool.tile([128, BLOC, 2, T], MM_DT, name="hT")

            for b in range(BLOC):
                for j in range(NJ):
                    u_t = sbuf.tile([128, NSUB, D], F32, tag="u_t", name="u_t")
                    nc.sync.dma_start(u_t, u_r[b, j])

                    for th in range(NTH):
                        t0 = j * TT + th * SC  # chunk start (abs time)
                        uTs = []
                        for k in range(2):
                            uT_ps = ps_ut.tile(
                                [128, SC], F32, tag="uT_ps", name="uT_ps"
                            )
                            for s_ in range(SC // 128):
                                nc.tensor.transpose(
                                    uT_ps[:, s_ * 128 : (s_ + 1) * 128],
                                    u_t[:, th * (SC // 128) + s_,
                                        k * 128 : (k + 1) * 128],
                                    ident,
                                )
                            uT_sb = sbuf.tile(
                                [128, SC], MM_DT, tag="uT_sb", bufs=4, name="uT_sb"
                            )
                            nc.scalar.copy(uT_sb, uT_ps)
                            uTs.append(uT_sb)

                        for m in range(2):
                            bu_ps = ps_bu.tile(
                                [128, SC], F32, tag="bu_ps", name="bu_ps"
                            )
                            for k in range(2):
                                nc.tensor.matmul(
                                    bu_ps,
                                    B_sb[:, k, m * 128 : (m + 1) * 128],
                                    uTs[k],
                                    start=(k == 0),
                                    stop=(k == 1),
                                )
                            init = (
                                h0c[:, m : m + 1]
                                if t0 == 0
                                else hT[:, b, m, t0 - 1 : t0]
                            )
                            nc.vector.tensor_tensor_scan(
                                hT[:, b, m, t0 : t0 + SC],
                                A_bc[:, m],
                                bu_ps,
                                init,
                                op0=mult,
                                op1=add,
                            )

                    y_sb = sbuf.tile([128, NSUB * D], F32, tag="y_sb", name="y_sb")
                    for half in range(NSUB // 2):
                        y_ps = ps_y.tile([128, 2 * D], F32, tag="y_ps", name="y_ps")
                        for i in range(2):
                            s_ = half * 2 + i
                            t0 = j * TT + s_ * 128
                            for k in range(2):
                                nc.tensor.matmul(
                                    y_ps[:, i * D : (i + 1) * D],
                                    hT[:, b, k, t0 : t0 + 128],
                                    C_sb[:, k],
                                    start=(k == 0),
                                    stop=(k == 1),
                                )
                        nc.scalar.copy(
                            y_sb[:, half * 2 * D : (half + 1) * 2 * D], y_ps
                        )
                    nc.sync.dma_start(
                        y_r[b, j], y_sb.rearrange("p (s o) -> p s o", s=NSUB)
                    )

    nc.compile()
    return nc


def _get_nc():
    if "nc" not in _CACHE:
        _CACHE["nc"] = _build_nc()
    return _CACHE["nc"]


def make_in_maps(inputs, A, B, C, h0):
    u = np.ascontiguousarray(np.asarray(inputs, dtype=np.float32))
    A2 = np.ascontiguousarray(np.asarray(A, np.float32).reshape(2, 128).T)
    h02 = np.ascontiguousarray(np.asarray(h0, np.float32).reshape(2, 128).T)
    Br = np.ascontiguousarray(np.asarray(B, np.float32).reshape(2, 128, D))
    Cr = np.ascontiguousarray(np.asarray(C, np.float32).reshape(2, 128, D))
    return [
        {
            "u": np.ascontiguousarray(u[c * BLOC : (c + 1) * BLOC]),
            "A": A2,
            "B": Br,
            "C": Cr,
            "h0": h02,
        }
        for c in range(NCORES)
    ]


def kernel(inputs, A, B, C, h0, _trace=False):
    nc = _get_nc()
    in_maps = make_in_maps(inputs, A, B, C, h0)
    res = bass_utils.run_bass_kernel_spmd(
        nc, in_maps, core_ids=list(range(NCORES)), trace=_trace
    )
    out = np.concatenate([r["y"] for r in res.results], axis=0)
    if _trace:
        _CACHE["last_result"] = res
    return out



# revision 2
# speedup vs baseline: 1.0875x; 1.0875x over previous
"""Trainium2 Bass kernel for a diagonal-A linear dynamical system (LDS).

    Bu = inputs @ B            [B, T, S]
    h_t = h_{t-1} * A + Bu_t   (scan over T, diagonal A)
    y_t = h_t @ C              [B, T, O]

Shapes: inputs [16, 4096, 256], A [256], B [256, 256], C [256, 256],
h0 [256]; all float32.

Sharding: data-parallel over batch across 8 NeuronCores (2 batches per
core); A/B/C/h0 replicated.

v2: u/B/C/hT in bf16 (u converted host-side -> halves DMA-in bytes and
makes the PE transposes single-pass with fast weight load). The scan
keeps fp32 internal state (DVE guarantees this) and bu stays fp32 in
PSUM, so only input/storage rounding is lost: measured rel err ~4e-3
vs the 2e-2 gate.

Per-core dataflow (all tiles 128-partition):
  1. DMA u supertile [128t, 4sub, 256i] bf16 (natural layout).
  2. PE transpose 128x128 blocks -> uT bf16 in PSUM (single pass).
  3. ACT copies uT PSUM->SBUF (bf16, 2x mode).
  4. PE matmul BuT[s, t] = B^T @ uT accumulated over i-halves into PSUM
     (fp32).
  5. DVE tensor_tensor_scan along t: state = A*state + Bu (fp32 internal
     state), chained across chunks via initial=prev last column. Output
     hT bf16 in SBUF.
  6. PE matmul y[t, o] = hT.T @ C (hT slices stationary, bf16 FWL).
  7. ACT copy y PSUM->SBUF fp32, DMA out.
"""

import ml_dtypes
import numpy as np

import concourse.bacc as bacc
import concourse.bass as bass
import concourse.mybir as mybir
import concourse.tile as tile
from concourse import bass_utils
from concourse.masks import make_identity

BATCH, T, D = 16, 4096, 256
NCORES = 8
BLOC = BATCH // NCORES  # batches per core
TT = 1024               # time supertile (DMA granularity)
NSUB = TT // 128        # 128-row subtiles per supertile
NJ = T // TT            # supertiles per sequence
SC = 512                # scan / PSUM chunk within a supertile
NTH = TT // SC          # chunks per supertile
F32 = mybir.dt.float32
BF16 = mybir.dt.bfloat16

_CACHE: dict = {}


def _build_nc():
    nc = bacc.Bacc(trn_type="TRN2", target_bir_lowering=False)

    u = nc.dram_tensor("u", [BLOC, T, D], BF16, kind="ExternalInput")
    Ad = nc.dram_tensor("A", [128, 2], F32, kind="ExternalInput")      # [s%128, s//128]
    Bd = nc.dram_tensor("B", [2, 128, D], BF16, kind="ExternalInput")  # [ihalf, i, s]
    Cd = nc.dram_tensor("C", [2, 128, D], BF16, kind="ExternalInput")  # [shalf, s, o]
    h0d = nc.dram_tensor("h0", [128, 2], F32, kind="ExternalInput")
    y = nc.dram_tensor("y", [BLOC, T, D], F32, kind="ExternalOutput")

    # t = j*TT + sub*128 + p
    u_r = u[:].rearrange("b (j s p) i -> b j p s i", p=128, s=NSUB)
    y_r = y[:].rearrange("b (j s p) o -> b j p s o", p=128, s=NSUB)

    mult = mybir.AluOpType.mult
    add = mybir.AluOpType.add

    bj = [(b, j) for b in range(BLOC) for j in range(NJ)]

    with tile.TileContext(nc) as tc:
        with (
            tc.tile_pool(name="const", bufs=1) as const,
            tc.tile_pool(name="upool", bufs=4) as upool,
            tc.tile_pool(name="sbuf", bufs=4) as sbuf,
            tc.tile_pool(name="hpool", bufs=1) as hpool,
            tc.tile_pool(name="ps_ut", bufs=2, space="PSUM") as ps_ut,
            tc.tile_pool(name="ps_bu", bufs=3, space="PSUM") as ps_bu,
            tc.tile_pool(name="ps_y", bufs=3, space="PSUM") as ps_y,
        ):
            # --- input prefetch first: get the DMA engines going early ---
            u_tiles = {}
            for b, j in bj:
                u_t = upool.tile([128, NSUB, D], BF16, tag="u_t", name="u_t")
                nc.sync.dma_start(u_t, u_r[b, j])
                u_tiles[(b, j)] = u_t

            # --- constants ---
            A_col = const.tile([128, 2], F32, name="A_col")
            nc.sync.dma_start(A_col, Ad[:])
            h0c = const.tile([128, 2], F32, name="h0c")
            nc.sync.dma_start(h0c, h0d[:])

            B_sb = const.tile([128, 2, D], BF16, name="B_sb")
            C_sb = const.tile([128, 2, D], BF16, name="C_sb")
            for k in range(2):
                nc.sync.dma_start(B_sb[:, k], Bd[k])
                nc.sync.dma_start(C_sb[:, k], Cd[k])

            ident = const.tile([128, 128], BF16, name="ident")
            make_identity(nc, ident)

            ones = const.tile([128, SC], F32, name="ones")
            nc.vector.memset(ones, 1.0)
            A_bc = const.tile([128, 2, SC], F32, name="A_bc")
            for m in range(2):
                nc.scalar.mul(A_bc[:, m], ones, mul=A_col[:, m : m + 1])

            # hidden states, [128s, b, mhalf, t]; persistent
            hT = hpool.tile([128, BLOC, 2, T], BF16, name="hT")

            for b, j in bj:
                u_t = u_tiles.pop((b, j))

                for th in range(NTH):
                    t0 = j * TT + th * SC  # chunk start (abs time)
                    uTs = []
                    for k in range(2):
                        uT_ps = ps_ut.tile(
                            [128, SC], BF16, tag="uT_ps", name="uT_ps"
                        )
                        for s_ in range(SC // 128):
                            nc.tensor.transpose(
                                uT_ps[:, s_ * 128 : (s_ + 1) * 128],
                                u_t[:, th * (SC // 128) + s_,
                                    k * 128 : (k + 1) * 128],
                                ident,
                            )
                        uT_sb = sbuf.tile(
                            [128, SC], BF16, tag="uT_sb", bufs=8, name="uT_sb"
                        )
                        nc.scalar.copy(uT_sb, uT_ps)
                        uTs.append(uT_sb)

                    for m in range(2):
                        bu_ps = ps_bu.tile(
                            [128, SC], F32, tag="bu_ps", name="bu_ps"
                        )
                        for k in range(2):
                            nc.tensor.matmul(
                                bu_ps,
                                B_sb[:, k, m * 128 : (m + 1) * 128],
                                uTs[k],
                                start=(k == 0),
                                stop=(k == 1),
                            )
                        init = (
                            h0c[:, m : m + 1]
                            if t0 == 0
                            else hT[:, b, m, t0 - 1 : t0]
                        )
                        nc.vector.tensor_tensor_scan(
                            hT[:, b, m, t0 : t0 + SC],
                            A_bc[:, m],
                            bu_ps,
                            init,
                            op0=mult,
                            op1=add,
                        )

                y_sb = sbuf.tile([128, NSUB * D], F32, tag="y_sb", name="y_sb")
                for half in range(NSUB // 2):
                    y_ps = ps_y.tile([128, 2 * D], F32, tag="y_ps", name="y_ps")
                    for i in range(2):
                        s_ = half * 2 + i
                        t0 = j * TT + s_ * 128
                        for k in range(2):
                            nc.tensor.matmul(
                                y_ps[:, i * D : (i + 1) * D],
                                hT[:, b, k, t0 : t0 + 128],
                                C_sb[:, k],
                                start=(k == 0),
                                stop=(k == 1),
                            )
                    nc.scalar.copy(
                        y_sb[:, half * 2 * D : (half + 1) * 2 * D], y_ps
                    )
                nc.sync.dma_start(
                    y_r[b, j], y_sb.rearrange("p (s o) -> p s o", s=NSUB)
                )

    nc.compile()
    return nc


def _get_nc():
    if "nc" not in _CACHE:
        _CACHE["nc"] = _build_nc()
    return _CACHE["nc"]


def make_in_maps(inputs, A, B, C, h0):
    u = np.ascontiguousarray(
        np.asarray(inputs, dtype=np.float32).astype(ml_dtypes.bfloat16)
    )
    A2 = np.ascontiguousarray(np.asarray(A, np.float32).reshape(2, 128).T)
    h02 = np.ascontiguousarray(np.asarray(h0, np.float32).reshape(2, 128).T)
    Br = np.ascontiguousarray(
        np.asarray(B, np.float32).reshape(2, 128, D).astype(ml_dtypes.bfloat16)
    )
    Cr = np.ascontiguousarray(
        np.asarray(C, np.float32).reshape(2, 128, D).astype(ml_dtypes.bfloat16)
    )
    return [
        {
            "u": np.ascontiguousarray(u[c * BLOC : (c + 1) * BLOC]),
            "A": A2,
            "B": Br,
            "C": Cr,
            "h0": h02,
        }
        for c in range(NCORES)
    ]


def kernel(inputs, A, B, C, h0, _trace=False):
    nc = _get_nc()
    in_maps = make_in_maps(inputs, A, B, C, h0)
    res = bass_utils.run_bass_kernel_spmd(
        nc, in_maps, core_ids=list(range(NCORES)), trace=_trace
    )
    out = np.concatenate([r["y"] for r in res.results], axis=0)
    if _trace:
        _CACHE["last_result"] = res
    return out


# revision 3
# speedup vs baseline: 1.1437x; 1.0517x over previous
"""Trainium2 Bass kernel for a diagonal-A linear dynamical system (LDS).

    Bu = inputs @ B            [B, T, S]
    h_t = h_{t-1} * A + Bu_t   (scan over T, diagonal A)
    y_t = h_t @ C              [B, T, O]

Shapes: inputs [16, 4096, 256], A [256], B [256, 256], C [256, 256],
h0 [256]; all float32.

Sharding: data-parallel over batch across 8 NeuronCores (2 batches per
core); A/B/C/h0 replicated.

v3: u/B/C/hT in bf16 (u converted host-side). u is transposed on the
way in by the DMA xbar (dma_start_transpose, needs 2-byte dtype), which
removes the PE transposes and the PSUM->SBUF copies of v1/v2 entirely.
The scan keeps fp32 internal state and Bu stays fp32 in PSUM; measured
rel err ~4e-3 vs the 2e-2 gate.

Per-core dataflow (all tiles 128-partition):
  1. DMA-transpose u [TT t, 128 i] bf16 HBM -> uT [128 i, TT t] SBUF.
  2. PE matmul BuT[s, t] = B^T @ uT accumulated over i-halves into PSUM
     (fp32).
  3. DVE tensor_tensor_scan along t: state = A*state + Bu (fp32 internal
     state), chained across chunks via initial=prev last column. Output
     hT bf16 in SBUF.
  4. PE matmul y[t, o] = hT.T @ C (hT slices stationary, bf16 FWL).
  5. ACT copy y PSUM->SBUF fp32, DMA out.
"""

import ml_dtypes
import numpy as np

import concourse.bacc as bacc
import concourse.bass as bass
import concourse.mybir as mybir
import concourse.tile as tile
from concourse import bass_utils

BATCH, T, D = 16, 4096, 256
NCORES = 8
BLOC = BATCH // NCORES  # batches per core
TT = 1024               # time supertile (DMA granularity)
NSUB = TT // 128        # 128-row subtiles per supertile
NJ = T // TT            # supertiles per sequence
SC = 512                # scan / PSUM chunk within a supertile
NTH = TT // SC          # chunks per supertile
F32 = mybir.dt.float32
BF16 = mybir.dt.bfloat16

_CACHE: dict = {}


def _build_nc():
    nc = bacc.Bacc(trn_type="TRN2", target_bir_lowering=False)

    u = nc.dram_tensor("u", [BLOC, T, D], BF16, kind="ExternalInput")
    Ad = nc.dram_tensor("A", [128, 2], F32, kind="ExternalInput")      # [s%128, s//128]
    Bd = nc.dram_tensor("B", [2, 128, D], BF16, kind="ExternalInput")  # [ihalf, i, s]
    Cd = nc.dram_tensor("C", [2, 128, D], BF16, kind="ExternalInput")  # [shalf, s, o]
    h0d = nc.dram_tensor("h0", [128, 2], F32, kind="ExternalInput")
    y = nc.dram_tensor("y", [BLOC, T, D], F32, kind="ExternalOutput")

    y_r = y[:].rearrange("b (j s p) o -> b j p s o", p=128, s=NSUB)

    mult = mybir.AluOpType.mult
    add = mybir.AluOpType.add

    bj = [(b, j) for b in range(BLOC) for j in range(NJ)]

    with tile.TileContext(nc) as tc:
        with (
            tc.tile_pool(name="const", bufs=1) as const,
            tc.tile_pool(name="upool", bufs=6) as upool,
            tc.tile_pool(name="sbuf", bufs=4) as sbuf,
            tc.tile_pool(name="hpool", bufs=1) as hpool,
            tc.tile_pool(name="ps_bu", bufs=4, space="PSUM") as ps_bu,
            tc.tile_pool(name="ps_y", bufs=4, space="PSUM") as ps_y,
        ):
            # --- input prefetch first: get the DMA engines going early.
            # The DMA xbar transposes [TT, 128] bf16 -> [128, TT] on the fly.
            uT_tiles = {}
            for b, j in bj:
                for k in range(2):
                    uT = upool.tile([128, TT], BF16, tag="uT", name="uT")
                    nc.sync.dma_start_transpose(
                        uT,
                        u[b, j * TT : (j + 1) * TT, k * 128 : (k + 1) * 128],
                    )
                    uT_tiles[(b, j, k)] = uT

            # --- constants ---
            A_col = const.tile([128, 2], F32, name="A_col")
            nc.sync.dma_start(A_col, Ad[:])
            h0c = const.tile([128, 2], F32, name="h0c")
            nc.sync.dma_start(h0c, h0d[:])

            B_sb = const.tile([128, 2, D], BF16, name="B_sb")
            C_sb = const.tile([128, 2, D], BF16, name="C_sb")
            for k in range(2):
                nc.sync.dma_start(B_sb[:, k], Bd[k])
                nc.sync.dma_start(C_sb[:, k], Cd[k])

            ones = const.tile([128, SC], F32, name="ones")
            nc.vector.memset(ones, 1.0)
            A_bc = const.tile([128, 2, SC], F32, name="A_bc")
            for m in range(2):
                nc.scalar.mul(A_bc[:, m], ones, mul=A_col[:, m : m + 1])

            # hidden states, [128s, b, mhalf, t]; persistent
            hT = hpool.tile([128, BLOC, 2, T], BF16, name="hT")

            for b, j in bj:
                uTs = [uT_tiles.pop((b, j, k)) for k in range(2)]

                for th in range(NTH):
                    t0 = j * TT + th * SC  # chunk start (abs time)
                    for m in range(2):
                        bu_ps = ps_bu.tile(
                            [128, SC], F32, tag="bu_ps", name="bu_ps"
                        )
                        for k in range(2):
                            nc.tensor.matmul(
                                bu_ps,
                                B_sb[:, k, m * 128 : (m + 1) * 128],
                                uTs[k][:, th * SC : (th + 1) * SC],
                                start=(k == 0),
                                stop=(k == 1),
                            )
                        init = (
                            h0c[:, m : m + 1]
                            if t0 == 0
                            else hT[:, b, m, t0 - 1 : t0]
                        )
                        nc.vector.tensor_tensor_scan(
                            hT[:, b, m, t0 : t0 + SC],
                            A_bc[:, m],
                            bu_ps,
                            init,
                            op0=mult,
                            op1=add,
                        )

                y_sb = sbuf.tile([128, NSUB * D], F32, tag="y_sb", name="y_sb")
                for half in range(NSUB // 2):
                    y_ps = ps_y.tile([128, 2 * D], F32, tag="y_ps", name="y_ps")
                    for i in range(2):
                        s_ = half * 2 + i
                        t0 = j * TT + s_ * 128
                        for k in range(2):
                            nc.tensor.matmul(
                                y_ps[:, i * D : (i + 1) * D],
                                hT[:, b, k, t0 : t0 + 128],
                                C_sb[:, k],
                                start=(k == 0),
                                stop=(k == 1),
                            )
                    nc.scalar.copy(
                        y_sb[:, half * 2 * D : (half + 1) * 2 * D], y_ps
                    )
                nc.sync.dma_start(
                    y_r[b, j], y_sb.rearrange("p (s o) -> p s o", s=NSUB)
                )

    nc.compile()
    return nc


def _get_nc():
    if "nc" not in _CACHE:
        _CACHE["nc"] = _build_nc()
    return _CACHE["nc"]


def make_in_maps(inputs, A, B, C, h0):
    u = np.ascontiguousarray(
        np.asarray(inputs, dtype=np.float32).astype(ml_dtypes.bfloat16)
    )
    A2 = np.ascontiguousarray(np.asarray(A, np.float32).reshape(2, 128).T)
    h02 = np.ascontiguousarray(np.asarray(h0, np.float32).reshape(2, 128).T)
    Br = np.ascontiguousarray(
        np.asarray(B, np.float32).reshape(2, 128, D).astype(ml_dtypes.bfloat16)
    )
    Cr = np.ascontiguousarray(
        np.asarray(C, np.float32).reshape(2, 128, D).astype(ml_dtypes.bfloat16)
    )
    return [
        {
            "u": np.ascontiguousarray(u[c * BLOC : (c + 1) * BLOC]),
            "A": A2,
            "B": Br,
            "C": Cr,
            "h0": h02,
        }
        for c in range(NCORES)
    ]


def kernel(inputs, A, B, C, h0, _trace=False):
    nc = _get_nc()
    in_maps = make_in_maps(inputs, A, B, C, h0)
    res = bass_utils.run_bass_kernel_spmd(
        nc, in_maps, core_ids=list(range(NCORES)), trace=_trace
    )
    out = np.concatenate([r["y"] for r in res.results], axis=0)
    if _trace:
        _CACHE["last_result"] = res
    return out
